# revision 2
# baseline (speedup 1.0000x reference)
"""Trainium2 Bass kernel for the CRF scoring module (nn_CRF_14379550507279).

reference math:
    score0      = transitions[tags[:,0]] + emissions[:,0]            # (B,T)
    trans_steps = transitions[tags[:,:-1], tags[:,1:]] * mask[:,1:]  # (B,S-1)
    emit_steps  = emissions[:,1:,:] * mask[:,1:,None]                # (B,S-1,T)
    total = score0.sum() + trans_steps.sum()*T + emit_steps.sum()

Everything is evaluated as PE matmuls over low-precision streams, one core per
batch shard (B=512 -> 8 x 64), host sums the 8 scalars:

  emissions (fp8e4m3, DoubleRow): psE[m,(c,t)] += sum_p sum_ko
      me[p,2(8j+m)+ko] * ems[p,2(8j+c)+ko,t]; the 8 diagonal (1,32) blocks of
      the (8,256) PSUM accumulator hold the masked emission sums. me rides in
      even/odd column planes so the dual-fp8 LDWEIGHTS restriction holds.
  pair histogram: C[i,j] = sum_c U[c-1,i]*U[c,j] with U = onehot(tags) SHARED
      between both matmul operands (stationary = moving shifted one column),
      4 column-pairs per (128,128) PSUM matmul, extraction via a
      block-diagonal 32*kron(I4, Tr) dot. One is_equal per column: the
      broadcast expansion (tags repeated 32x) runs on the otherwise-idle
      Activation/GPSIMD engines so the DVE is_equal hits its 2x bf16 mode.
      Mask corrections (the ~10% zero-mask pairs, host-compacted) are
      host-encoded one-hot planes (prev side = -1.0, fp8) whose matmuls
      accumulate NEGATIVELY into the same PSUM tile. score0 and the
      column-0 pairs fold into the same accumulator (count0 x ones/32).

Input DMAs are minimized and split across both HWDGE rings (per-DMA fixed
cost and the contended ~180GB/s per-core HBM rate dominate on 8 busy cores).
"""
import numpy as np
import ml_dtypes

import concourse.bass as bass
import concourse.bacc as bacc
import concourse.mybir as mybir
import concourse.tile as tile
from concourse.bass_utils import run_bass_kernel_spmd

F32 = mybir.dt.float32
BF16 = mybir.dt.bfloat16
FP8 = mybir.dt.float8e4
ALU = mybir.AluOpType
ACTF = mybir.ActivationFunctionType
AXL = mybir.AxisListType

N_CORES = 8
B, S, T = 512, 2048, 32
BC = B // N_CORES          # 64 batches per core
P = 128                    # SBUF partitions
RPP = BC * S // P          # 1024 step-columns per partition
NCH = 4                    # emissions chunks
OG = 68                    # max step-columns per one-hot build group
GSZ = [68] * 15 + [4]      # last group tiny so the tail starts early
GOF = [sum(GSZ[:g]) for g in range(len(GSZ))]
NG = len(GSZ)
NCR = 160                  # compacted correction columns (>= worst partition)

# tgx: [pad(1,-3) | tags(RPP) | pm0 | nxt0 | tg0]
O_TAGS = 1
O_X0 = O_TAGS + RPP
TGX_W = O_X0 + 3

_cached = {}


def _build(repeat=1, serialize=False, do_emis=True, do_hist=True, do_s0=True,
           do_corr=True, nch=NCH, dr=True):
    nc = bacc.Bacc("TRN2", target_bir_lowering=False, debug=False)

    # f8 block: [men(RPP) | ems(RPP*T)]
    f8d = nc.dram_tensor("f8d", [P, RPP + RPP * T], FP8, kind="ExternalInput")
    # b16 block: [pad(1,-3) | tags(RPP) | x0(3)]
    b16d = nc.dram_tensor("b16d", [P, TGX_W], BF16, kind="ExternalInput")
    cohd = nc.dram_tensor("cohd", [P, 2 * NCR * T], FP8, kind="ExternalInput")
    # [wbl(P) | id16(16) | trt(T)]
    cons = nc.dram_tensor("cons", [P, P + 16 + T], F32, kind="ExternalInput")
    out = nc.dram_tensor("out", [1, 1], F32, kind="ExternalOutput")

    exp_eng = {}
    for g in range(NG):
        if g in (0, 15):
            exp_eng[g] = None
        elif g in (2, 5, 8, 11, 14):
            exp_eng[g] = "pool"
        else:
            exp_eng[g] = "act"
    with tile.TileContext(nc) as tc:
        with (
            tc.tile_pool(name="pers", bufs=1) as pers,
            tc.tile_pool(name="epool", bufs=8) as epool,
            tc.tile_pool(name="upool", bufs=8) as upool,
            tc.tile_pool(name="xpool", bufs=6) as xpool,
            tc.tile_pool(name="psum", bufs=1, space="PSUM") as psump,
        ):
          for _rep in range(repeat):
            # ---------- input loads: all on the SP ring; ACT only expands
            b16_t = pers.tile([P, TGX_W], BF16, tag="b16")
            nc.sync.dma_start(b16_t[:], b16d[:])
            tgx_t = b16_t[:, 0:TGX_W]
            coh_t = pers.tile([P, 2 * NCR * T], FP8, tag="coh")
            coh = coh_t[:]
            iot_t = pers.tile([P, (OG + 1) * T], BF16, tag="iott")
            nc.gpsimd.iota(iot_t[:].rearrange("p (s t) -> p s t", t=T),
                           pattern=[[0, OG + 1], [1, T]], base=0,
                           channel_multiplier=0,
                           allow_small_or_imprecise_dtypes=True)
            iot = iot_t[:]
            iot2 = iot

            onesf = pers.tile([P, 1], F32, tag="onesf")
            nc.vector.memset(onesf[:], 1.0)

            # emissions chunk loads (men rides with chunk 0), then coh, cons
            cch = RPP // nch
            f8_t = pers.tile([P, RPP + RPP * T], FP8, tag="f8")
            men_t = f8_t[:, 0:RPP]
            ets = []
            if do_emis:
                for j in range(nch):
                    lo = RPP + j * cch * T if j > 0 else 0
                    hi = RPP + (j + 1) * cch * T
                    dma_eng = nc.scalar if j % 2 == 0 else nc.sync
                    dma_eng.dma_start(f8_t[:, lo:hi], f8d[:, lo:hi])
                    ets.append(f8_t[:, RPP + j * cch * T:RPP + (j + 1) * cch * T])
            if do_corr:
                nc.scalar.dma_start(coh_t[:], cohd[:])
            cons_t = pers.tile([P, P + 16 + T], F32, tag="cons")
            nc.sync.dma_start(cons_t[:], cons[:])
            wbl_t = cons_t[:, 0:P]
            id16 = cons_t[0:16, P:P + 16]

            # ---------- extras (col-0 pairs + score0) fold into psC:
            # pair-hist outer products land in the (0,0) Tr-weighted block;
            # score0 = count0 x (ones/32) there too (x32 is baked into wbl).
            i32h = pers.tile([P, T], BF16, tag="i32h")
            nc.vector.memset(i32h[:], 1.0 / 32.0)

            def emit_extras_a():
                xoh = pers.tile([P, 3 * T], BF16, tag="xoh")
                nc.vector.tensor_tensor(
                    xoh[:].rearrange("p (c t) -> p c t", t=T),
                    tgx_t[:, O_X0:O_X0 + 3].broadcast_to((P, 3, T)),
                    iot[:, 0:3 * T].rearrange("p (c t) -> p c t", t=T),
                    ALU.is_equal)
                nc.tensor.matmul(psC[0:T, 0:T], xoh[:, 0:T], xoh[:, T:2 * T],
                                 start=False, stop=False,
                                 skip_group_check=True)
                nc.tensor.matmul(psC[0:T, 0:T], xoh[:, 2 * T:3 * T],
                                 i32h[:], start=False, stop=False,
                                 skip_group_check=True)

            # ---------- main interleaved stream ----------
            psE = psump.tile([16, 512], F32, tag="psE")
            psC = psump.tile([P, P], F32, tag="psC")
            efin = pers.tile([16, 1], F32, tag="efin")
            for j in range(nch):
                if do_hist:
                    for g in range(NG * j // nch, NG * (j + 1) // nch):
                        sz = GSZ[g]
                        nmm = sz // 4
                        U = upool.tile([P, (OG + 1) * T], BF16, tag="U")
                        eng = exp_eng[g]
                        if eng is None:
                            nc.vector.tensor_tensor(
                                U[:, 0:(sz + 1) * T]
                                    .rearrange("p (s t) -> p s t", t=T),
                                tgx_t[:, GOF[g]:GOF[g] + sz + 1]
                                    .broadcast_to((P, sz + 1, T)),
                                iot[:, 0:(sz + 1) * T]
                                    .rearrange("p (s t) -> p s t", t=T),
                                ALU.is_equal)
                        else:
                            xg = xpool.tile([P, (OG + 1) * T], BF16, tag="xg")
                            src_ap = tgx_t[:, GOF[g]:GOF[g] + sz + 1] \
                                .rearrange("p s -> p s ()") \
                                .broadcast_to((P, sz + 1, T))
                            dst_ap = xg[:, 0:(sz + 1) * T] \
                                .rearrange("p (s t) -> p s t", t=T)
                            if eng == "act":
                                nc.scalar.activation(dst_ap, src_ap, ACTF.Copy)
                            else:
                                nc.gpsimd.tensor_copy(dst_ap, src_ap)
                            nc.vector.tensor_tensor(
                                U[:, 0:(sz + 1) * T], xg[:, 0:(sz + 1) * T],
                                iot2[:, 0:(sz + 1) * T], ALU.is_equal)
                        # corrections join the same accumulation, negatively,
                        # just before the last group
                        if do_corr and g == NG - 1:
                            for c in range(NCR // 4):
                                nc.tensor.matmul(
                                    psC[:],
                                    coh[:, 4 * c * T:(4 * c + 4) * T],
                                    coh[:, (NCR + 4 * c) * T:
                                         (NCR + 4 * c + 4) * T],
                                    start=False, stop=False,
                                    skip_group_check=True)
                        for c in range(nmm):
                            nc.tensor.matmul(
                                psC[:],
                                U[:, 4 * c * T:(4 * c + 4) * T],
                                U[:, (4 * c + 1) * T:(4 * c + 5) * T],
                                start=(g == 0 and c == 0),
                                stop=(g == NG - 1 and c == nmm - 1),
                                skip_group_check=True)
                        if do_s0 and g == 0:
                            emit_extras_a()
                if do_emis:
                    et = ets[j]
                    for k in range(cch // 16):
                        if dr:
                            pos = (j * cch + 16 * k) // 2
                            nc.tensor.matmul(
                                psE[0:8, 0:256],
                                men_t.rearrange("p (ko m) -> p ko m", ko=2)
                                    [:, :, pos:pos + 8],
                                et[:, 512 * k:512 * k + 512]
                                    .rearrange("p (cp ko t) -> p ko cp t",
                                               ko=2, t=T),
                                start=(j == 0 and k == 0),
                                stop=(j == nch - 1 and k == cch // 16 - 1),
                                perf_mode=mybir.MatmulPerfMode.DoubleRow,
                                skip_group_check=True)
                        else:
                            nc.tensor.matmul(
                                psE[:],
                                men_t[:, j * cch + 16 * k:j * cch + 16 * k + 16],
                                et[:, 512 * k:512 * k + 512],
                                start=(j == 0 and k == 0),
                                stop=(j == nch - 1 and k == cch // 16 - 1),
                                skip_group_check=True)
                    if j == nch - 1:
                        # emissions extraction, mid-stream
                        nd = 8 if dr else 16
                        ec = pers.tile([16, 512], F32, tag="ec")
                        nc.vector.tensor_copy(ec[0:nd, 0:32 * nd],
                                              psE[0:nd, 0:32 * nd])
                        e2 = pers.tile([16, 16], F32, tag="e2")
                        nc.vector.tensor_reduce(
                            e2[0:nd, 0:nd],
                            ec[0:nd, 0:32 * nd]
                                .rearrange("p (g t) -> p g t", t=T),
                            axis=AXL.X, op=ALU.add)
                        nc.vector.tensor_tensor(e2[0:nd, 0:nd], e2[0:nd, 0:nd],
                                                id16[0:nd, 0:nd], ALU.mult)
                        nc.vector.tensor_reduce(efin[0:nd], e2[0:nd, 0:nd],
                                                axis=AXL.X, op=ALU.add)

            # ---------- tail: fused histogram extraction + final dots ----
            psF = psump.tile([1, 1], F32, tag="psF")
            hfin = pers.tile([P, 1], F32, tag="hfin")
            if do_hist:
                hc = pers.tile([P, P], F32, tag="hc")
                nc.vector.tensor_tensor(hc[:], psC[:], wbl_t, ALU.mult)
                nc.vector.tensor_reduce(hfin[:], hc[:], axis=AXL.X, op=ALU.add)
            else:
                nc.vector.memset(hfin[:], 0.0)
            nc.tensor.matmul(psF[:], hfin[:], onesf[:], start=True,
                             stop=not do_emis, skip_group_check=True)
            if do_emis:
                nd = 8 if dr else 16
                nc.tensor.matmul(psF[:], efin[0:nd], onesf[0:nd], start=False,
                                 stop=True, skip_group_check=True)
            osb = pers.tile([1, 1], F32, tag="osb")
            nc.vector.tensor_copy(osb[:], psF[:])
            nc.sync.dma_start(out[:], osb[:])
            if serialize:
                tc.strict_bb_all_engine_barrier()
    nc.compile()
    return nc


def _in_maps(emissions, tags, mask, transitions):
    bf = ml_dtypes.bfloat16
    E = np.asarray(emissions, np.float32)
    tg = np.asarray(tags, np.int32)
    mk = np.asarray(mask, np.float32)
    tr = np.asarray(transitions, np.float32)

    me = mk.copy()
    me[:, 0] = 1.0                        # emission col 0 is unmasked

    iot = np.ascontiguousarray(np.broadcast_to(
        np.arange(T, dtype=np.float32)[None, None, :], (P, OG + 1, T))
    ).astype(bf).reshape(P, (OG + 1) * T)
    wbl = 32.0 * np.kron(np.eye(4, dtype=np.float32), tr)
    id16 = np.zeros((P, 16), np.float32)
    id16[0:16, 0:16] = np.eye(16, dtype=np.float32)
    trtp = np.zeros((P, T), np.float32)
    trtp[0:T] = tr
    cons = np.ascontiguousarray(
        np.concatenate([wbl, id16, trtp], axis=1)).astype(np.float32)

    ar32 = np.arange(T, dtype=np.float32)
    parity = (np.arange(P) % 2 == 0)
    maps = []
    for c in range(N_CORES):
        sl_ = slice(c * BC, (c + 1) * BC)
        tgc = tg[sl_].reshape(P, RPP)
        mkc = mk[sl_].reshape(P, RPP)
        cprev = np.full((P, NCR), -1.0, np.float32)
        cnext = np.full((P, NCR), -2.0, np.float32)
        for p in range(P):
            idx = np.nonzero(mkc[p, 1:] == 0.0)[0] + 1
            assert len(idx) <= NCR, f"NCR too small: {len(idx)}"
            cprev[p, :len(idx)] = tgc[p, idx - 1]
            cnext[p, :len(idx)] = tgc[p, idx]
        # prev plane carries -1.0 so these matmuls SUBTRACT from the psum
        coh = np.concatenate([
            -(cprev[:, :, None] == ar32).astype(np.float32),
            (cnext[:, :, None] == ar32).astype(np.float32),
        ], axis=1).astype(bf).reshape(P, 2 * NCR * T)
        prev0 = np.empty(P, np.float32)
        prev0[1:] = tgc[:-1, RPP - 1]
        prev0[0] = -1.0
        w0 = mkc[:, 0].copy()
        w0[parity] = 0.0
        pm0 = (prev0 + 1.0) * w0 - 1.0
        nxt0 = tgc[:, 0].astype(np.float32)
        tg0 = np.where(parity, np.repeat(tg[sl_, 0], 2).astype(np.float32),
                       -1.0)
        tgx_c = np.concatenate([
            np.full((P, 1), -3.0, np.float32),
            tgc.astype(np.float32),
            pm0[:, None], nxt0[:, None], tg0[:, None]], axis=1).astype(bf)
        assert tgx_c.shape[1] == TGX_W
        f8 = ml_dtypes.float8_e4m3
        me_c = me[sl_].reshape(P, RPP)
        me_dr = np.concatenate([me_c[:, 0::2], me_c[:, 1::2]], axis=1)
        f8blk = np.concatenate([
            me_dr,
            E[sl_].reshape(P, RPP * T)], axis=1).astype(f8)
        b16blk = np.asarray(tgx_c).astype(bf)
        maps.append(dict(
            f8d=np.ascontiguousarray(f8blk),
            b16d=np.ascontiguousarray(b16blk),
            cohd=np.ascontiguousarray(np.asarray(coh, np.float32).astype(f8)),
            cons=cons,
        ))
    return maps


def kernel(emissions, tags, mask, transitions):
    if "nc" not in _cached:
        _cached["nc"] = _build()
    nc = _cached["nc"]
    maps = _in_maps(emissions, tags, mask, transitions)
    res = run_bass_kernel_spmd(nc, maps, list(range(N_CORES)))
    total = np.float64(0.0)
    for c in range(N_CORES):
        total += np.float64(res.results[c]["out"][0, 0])
    return np.float32(total)
